# revision 1
# baseline (speedup 1.0000x reference)
"""Trainium2 Bass kernel for a pre-LN transformer block (B=4, T=2048, D=1024,
H=16, HS=64, FF=4096, causal attention).

Sharding: data-parallel over batch pairs x 2-way tensor-parallel
(heads for attention, columns/rows for FFN) with a pair AllReduce after the
attention output projection and after FC2 (Megatron style).

Core c (0..7): batch b = c//2, TP half = c%2 (8 local heads, 2048 local FF).
All activations live feature-major on chip (d on partitions, t on free dim);
the host transposes x in and the output back.
"""

import numpy as np
import ml_dtypes

import concourse.bacc as bacc
import concourse.bass as bass
import concourse.mybir as mybir
import concourse.tile as tile
from concourse.bass_utils import run_bass_kernel_spmd

BF16NP = ml_dtypes.bfloat16

B, T, D, H, HS, FF = 4, 2048, 1024, 16, 64, 4096
EPS = 1e-5
NCORES = 8
TP = 2
LH = H // TP          # 8 local heads
LHE = LH * HS         # 512 local head-embed width
LFF = FF // TP        # 2048 local FF
KD = D // 128         # 8 d k-tiles
KHE = LHE // 128      # 4 he k-tiles
KFF = LFF // 128      # 16 ff k-tiles
NCH = T // 512        # 4 t-chunks of 512
NST = T // 128        # 16 s-tiles of 128
PAIRS = [[0, 1], [2, 3], [4, 5], [6, 7]]

F32 = mybir.dt.float32
BF = mybir.dt.bfloat16


def _emit(nc, tc, t):
    mm = nc.tensor.matmul
    Alu = mybir.AluOpType
    Act = mybir.ActivationFunctionType

    xT_v = t["xT"].rearrange("(k p) t -> p k t", p=128)
    w1_v = t["w1"].rearrange("(k p) e -> p k e", p=128)
    w2_v = t["w2"].rearrange("(k p) e -> p k e", p=128)
    outT_v = t["outT"]

    # ---------------- persistent pools (LIFO stack bottom) ----------------
    dram = tc.alloc_tile_pool(name="dram", bufs=1, space="DRAM")
    ar1_in = [dram.tile([D, 512], F32, name=f"ar1i{c}") for c in range(NCH)]
    ar1_out = [dram.tile([D, 512], F32, name=f"ar1o{c}") for c in range(NCH)]
    ar2_in = [dram.tile([D, 512], F32, name=f"ar2i{c}") for c in range(NCH)]
    ar2_out = [dram.tile([D, 512], F32, name=f"ar2o{c}") for c in range(NCH)]

    consts = tc.alloc_tile_pool(name="consts", bufs=1)
    ones_col = consts.tile([128, 1], BF)        # lhsT for column-sum matmuls
    nc.vector.memset(ones_col, 1.0)

    # per-d-row parameter vectors, [128, KD] layout: [p, k] = v[k*128+p]
    g1_sb = consts.tile([128, KD], F32)
    be1_sb = consts.tile([128, KD], F32)
    g2_sb = consts.tile([128, KD], F32)
    be2_sb = consts.tile([128, KD], F32)
    bo_sb = consts.tile([128, KD], F32)
    b2_sb = consts.tile([128, KD], F32)
    b1_sb = consts.tile([128, KFF], F32)
    for name, dst in (("g1", g1_sb), ("be1", be1_sb), ("g2", g2_sb),
                      ("be2", be2_sb), ("bo", bo_sb), ("b2", b2_sb)):
        nc.sync.dma_start(out=dst, in_=t[name].rearrange("(k p) -> p k", p=128))
    nc.sync.dma_start(out=b1_sb, in_=t["b1l"].rearrange("(k p) -> p k", p=128))

    # causal masks for the 4 diagonal offsets: 1 where t_rel-s_rel-128*m>=0
    masks = []
    for midx in range(4):
        mk = consts.tile([128, 512], BF, name=f"mask{midx}")
        nc.vector.memset(mk, 1.0)
        nc.gpsimd.affine_select(
            out=mk, in_=mk, compare_op=Alu.is_ge, fill=0.0,
            base=-(midx * 128), channel_multiplier=-1, pattern=[[1, 512]])
        masks.append(mk)

    # wo + oT live until stage D
    wlate = tc.alloc_tile_pool(name="wlate", bufs=1)
    wo_sb = wlate.tile([128, KHE, D], BF, tag="wo")
    nc.sync.dma_start(out=wo_sb, in_=t["wo"].rearrange("(k p) e -> p k e", p=128))
    oT = [wlate.tile([128, KHE, 512], BF, name=f"oT{c}") for c in range(NCH)]

    # ========== Stages A+B+C merged, software-pipelined per chunk ==========
    with tc.tile_pool(name="abc", bufs=1) as ab:
        hT = [ab.tile([128, KD, 512], BF, name=f"hT{c}") for c in range(NCH)]
        kT = ab.tile([128, LH // 2, T], BF, tag="kT")
        qT = ab.tile([128, LH // 2, T], BF, tag="qT")
        # v rows with an interleaved ones column per head: [s, 8*(64+1)]
        vS = ab.tile([128, NST, LH * 65], BF, tag="vS")
        nc.vector.memset(vS, 1.0)
        wq_sb = ab.tile([128, KD, LHE], BF, tag="wq")
        wk_sb = ab.tile([128, KD, LHE], BF, tag="wk")
        wv_sb = ab.tile([128, KD, LHE], BF, tag="wv")
        for src, dst in ((t["wq"], wq_sb), (t["wk"], wk_sb), (t["wv"], wv_sb)):
            nc.sync.dma_start(out=dst,
                              in_=src.rearrange("(k p) e -> p k e", p=128))

        pools = {}

        def ln1(ci):
            c0 = ci * 512
            xf = [ab.tile([128, 512], F32, tag="xf", bufs=9, name="xf")
                  for _ in range(KD)]
            for k in range(KD):
                nc.sync.dma_start(out=xf[k], in_=xT_v[:, k, c0:c0 + 512])
            Ab, Bb = _ln_stats(nc, tc, ab, pools["pstat"], xf, ones_col, F32)
            for k in range(KD):
                _ln_apply(nc, ab, xf[k], Ab, Bb, g1_sb, be1_sb, k,
                          hT[ci][:, k, :], F32)

        def proj(ci):
            c0 = ci * 512
            # k / q projections for this chunk
            for w_sb, dst in ((wk_sb, kT), (wq_sb, qT)):
                for et in range(LH // 2):
                    ps = pools["pproj"].tile([128, 512], F32, tag="ps_proj", bufs=2,
                                    name="ps_proj")
                    for k in range(KD):
                        mm(out=ps, lhsT=w_sb[:, k, et * 128:(et + 1) * 128],
                           rhs=hT[ci][:, k, :],
                           start=(k == 0), stop=(k == KD - 1))
                    nc.vector.tensor_copy(
                        out=dst[:, et, c0:c0 + 512], in_=ps)
            # v projection (row-major, into the 65-strided layout)
            for sti in range(4):
                st = ci * 4 + sti
                ps = pools["pproj"].tile([128, LHE], F32, tag="ps_proj", bufs=2,
                                name="ps_v")
                for k in range(KD):
                    mm(out=ps, lhsT=hT[ci][:, k, sti * 128:sti * 128 + 128],
                       rhs=wv_sb[:, k, :],
                       start=(k == 0), stop=(k == KD - 1))
                nc.vector.tensor_copy(
                    out=vS[:, st, :].rearrange("p (h e) -> p h e",
                                               h=LH)[:, :, 0:64],
                    in_=ps.rearrange("p (h e) -> p h e", e=64))

        def attention(ci):
            c0 = ci * 512
            nb = 4 * (ci + 1)
            dn8 = ab.tile([LH, 512], F32, tag="dn8", bufs=1, name="dn8")
            o_us = {}
            for hp in range(LH // 2):
                po = [pools["poa"].tile([65, 512], F32, tag="po", bufs=2, name="po")
                      for _ in range(2)]
                for sb in range(nb):
                    s0 = sb * 128
                    pss = [pools["psc"].tile([128, 512], F32, tag="ps_sc", bufs=4,
                                    name="ps_sc") for _ in range(2)]
                    for hi in range(2):
                        e0 = hi * 64
                        mm(out=pss[hi],
                           lhsT=kT[e0:e0 + 64, hp, s0:s0 + 128],
                           rhs=qT[e0:e0 + 64, hp, c0:c0 + 512],
                           start=True, stop=True)
                    exs = []
                    for hi in range(2):
                        ex = ab.tile([128, 512], BF, tag="ex", bufs=3,
                                     name="ex")
                        nc.scalar.activation(out=ex, in_=pss[hi],
                                             func=Act.Exp,
                                             scale=float(HS) ** -0.5)
                        exs.append(ex)
                    midx = sb - 4 * ci
                    if midx >= 0:
                        for hi in range(2):
                            nc.vector.tensor_mul(out=exs[hi], in0=exs[hi],
                                                 in1=masks[midx])
                    for hi in range(2):
                        h_loc = hp * 2 + hi
                        mm(out=po[hi],
                           lhsT=vS[:, sb, h_loc * 65:h_loc * 65 + 65],
                           rhs=exs[hi],
                           start=(sb == 0), stop=(sb == nb - 1))
                for hi in range(2):
                    h_loc = hp * 2 + hi
                    # unnormalized o (bf16) kept until batched reciprocal
                    ou = ab.tile([64, 512], BF, tag="ou", bufs=8,
                                 name="ou")
                    o_us[h_loc] = ou
                    nc.vector.tensor_copy(out=ou, in_=po[hi][0:64, :])
                    dnr = ab.tile([1, 512], F32, tag="dnr", bufs=1,
                                  name="dnr")
                    nc.vector.tensor_copy(out=dnr, in_=po[hi][64:65, :])
                    nc.sync.dma_start(out=dn8[h_loc:h_loc + 1, :], in_=dnr)
            rec8 = ab.tile([LH, 512], F32, tag="rec8", bufs=1, name="rec8")
            nc.vector.reciprocal(out=rec8, in_=dn8)
            rb8 = ab.tile([LH, 512], BF, tag="rb8", bufs=2, name="rb8")
            nc.vector.tensor_copy(out=rb8, in_=rec8)
            for h_loc in range(LH):
                rbt = ab.tile([1, 512], BF, tag="rbt", bufs=1, name="rbt")
                nc.sync.dma_start(out=rbt, in_=rb8[h_loc:h_loc + 1, :])
                bc = ab.tile([64, 512], BF, tag="bc", bufs=1, name="bc")
                nc.gpsimd.partition_broadcast(bc, rbt)
                nc.vector.tensor_mul(
                    out=oT[ci][(h_loc % 2) * 64:(h_loc % 2) * 64 + 64,
                               h_loc // 2, :],
                    in0=o_us[h_loc], in1=bc)

        def wo_ar1(ci):
            for dt in range(KD):
                ps = pools["pproj"].tile([128, 512], F32, tag="ps_proj",
                                         bufs=2, name="ps_wo")
                for k in range(KHE):
                    mm(out=ps, lhsT=wo_sb[:, k, dt * 128:(dt + 1) * 128],
                       rhs=oT[ci][:, k, :],
                       start=(k == 0), stop=(k == KHE - 1))
                stg = ab.tile([128, 512], F32, tag="stg1", bufs=2,
                              name="stg1")
                nc.scalar.copy(out=stg, in_=ps)
                nc.sync.dma_start(
                    out=ar1_in[ci][dt * 128:(dt + 1) * 128, :], in_=stg)
            nc.gpsimd.collective_compute(
                "AllReduce", Alu.add, replica_groups=PAIRS,
                ins=[ar1_in[ci].opt()], outs=[ar1_out[ci].opt()])

        # LN1 for all chunks first (frees the stats psum banks), then
        # software pipeline: projections of chunk ci overlap attention ci-1
        with tc.tile_pool(name="statpsum", bufs=1, space="PSUM") as pstat_:
            pools["pstat"] = pstat_
            for ci in range(NCH):
                ln1(ci)
        with tc.tile_pool(name="projpsum", bufs=2, space="PSUM") as pproj_, \
             tc.tile_pool(name="scpsum", bufs=4, space="PSUM") as psc_, \
             tc.tile_pool(name="oaccpsum", bufs=2, space="PSUM") as poa_:
            pools["pproj"] = pproj_
            pools["psc"] = psc_
            pools["poa"] = poa_
            for ci in range(NCH + 1):
                if ci < NCH:
                    proj(ci)
                if ci >= 1:
                    attention(ci - 1)
                    wo_ar1(ci - 1)

    # ========== Stage D1: Wo partials + AllReduce for all chunks ==========
    with tc.tile_pool(name="de", bufs=1) as de, \
         tc.tile_pool(name="ln2psum", bufs=1, space="PSUM") as pstat2, \
         tc.tile_pool(name="upsum", bufs=3, space="PSUM") as pu, \
         tc.tile_pool(name="fpsum", bufs=3, space="PSUM") as pf:

        # ========== Stage D2+E per chunk ==========
        for ci in range(NCH):
            c0 = ci * 512
            # residual 1: xmid = x + attn + bo  (bf16 residual stream)
            xmid = de.tile([128, KD, 512], BF, tag="xmid", bufs=2,
                           name="xmid")
            ar1v = ar1_out[ci].rearrange("(k p) t -> p k t", p=128)
            for k in range(KD):
                ar_sb = de.tile([128, 512], F32, tag="ar1sb", bufs=2,
                                name="ar1sb")
                nc.sync.dma_start(out=ar_sb, in_=ar1v[:, k, :])
                xf2 = de.tile([128, 512], F32, tag="xf2", bufs=2, name="xf2")
                nc.sync.dma_start(out=xf2, in_=xT_v[:, k, c0:c0 + 512])
                nc.vector.scalar_tensor_tensor(
                    out=xmid[:, k, :], in0=ar_sb,
                    scalar=bo_sb[:, k:k + 1], in1=xf2,
                    op0=Alu.add, op1=Alu.add)

            # LN2 (xmid already bf16: feed matmuls directly)
            src = [xmid[:, k, :] for k in range(KD)]
            Ab2, Bb2 = _ln_stats(nc, tc, de, pstat2, src, ones_col, BF)
            h2 = de.tile([128, KD, 512], BF, tag="h2", bufs=2, name="h2")
            for k in range(KD):
                _ln_apply(nc, de, src[k], Ab2, Bb2, g2_sb, be2_sb, k,
                          h2[:, k, :], BF)

            # FFN up: u = relu(h2 @ W1 + b1)
            u = de.tile([128, KFF, 512], BF, tag="u", bufs=2, name="u")
            for fp in range(KFF // 2):
                w1t = de.tile([128, KD, 256], BF, tag="w1t", bufs=2,
                              name="w1t")
                nc.sync.dma_start(out=w1t,
                                  in_=w1_v[:, :, fp * 256:(fp + 1) * 256])
                for half in range(2):
                    fft = fp * 2 + half
                    ps = pu.tile([128, 512], F32, tag="ps_u", bufs=3,
                                 name="ps_u")
                    for k in range(KD):
                        mm(out=ps,
                           lhsT=w1t[:, k, half * 128:half * 128 + 128],
                           rhs=h2[:, k, :],
                           start=(k == 0), stop=(k == KD - 1))
                    nc.scalar.activation(out=u[:, fft, :], in_=ps,
                                         func=Act.Relu,
                                         bias=b1_sb[:, fft:fft + 1])
            # FFN down partial -> AllReduce
            w2a = de.tile([128, KFF // 2, D], BF, tag="w2t", bufs=2,
                          name="w2a")
            w2b = de.tile([128, KFF // 2, D], BF, tag="w2t", bufs=2,
                          name="w2b")
            nc.sync.dma_start(out=w2a, in_=w2_v[:, 0:KFF // 2, :])
            nc.sync.dma_start(out=w2b, in_=w2_v[:, KFF // 2:KFF, :])
            for dt in range(KD):
                ps = pf.tile([128, 512], F32, tag="ps_f", bufs=3,
                             name="ps_f")
                for k2 in range(KFF):
                    wt = w2a if k2 < KFF // 2 else w2b
                    mm(out=ps,
                       lhsT=wt[:, k2 % (KFF // 2),
                               dt * 128:(dt + 1) * 128],
                       rhs=u[:, k2, :],
                       start=(k2 == 0), stop=(k2 == KFF - 1))
                stg = de.tile([128, 512], F32, tag="stg2", bufs=3,
                              name="stg2")
                nc.scalar.copy(out=stg, in_=ps)
                nc.sync.dma_start(
                    out=ar2_in[ci][dt * 128:(dt + 1) * 128, :], in_=stg)
            nc.gpsimd.collective_compute(
                "AllReduce", Alu.add, replica_groups=PAIRS,
                ins=[ar2_in[ci].opt()], outs=[ar2_out[ci].opt()])
            # residual 2 + store
            ar2v = ar2_out[ci].rearrange("(k p) t -> p k t", p=128)
            for dt in range(KD):
                ar2_sb = de.tile([128, 512], F32, tag="ar2sb", bufs=2,
                                 name="ar2sb")
                nc.sync.dma_start(out=ar2_sb, in_=ar2v[:, dt, :])
                o_f = de.tile([128, 512], F32, tag="o_f", bufs=2, name="o_f")
                nc.vector.scalar_tensor_tensor(
                    out=o_f, in0=ar2_sb,
                    scalar=b2_sb[:, dt:dt + 1], in1=xmid[:, dt, :],
                    op0=Alu.add, op1=Alu.add)
                nc.sync.dma_start(
                    out=outT_v[dt * 128:(dt + 1) * 128, c0:c0 + 512],
                    in_=o_f)

    # release persistent pools in reverse stack order
    wlate.release()
    consts.release()
    dram.release()


def _ln_stats(nc, tc, pool, pstat, src_tiles, ones_col, in_dt):
    """src_tiles: list of KD [128,512] APs (f32 or bf16) for one t-chunk.
    Returns (Ab, Bb) broadcast tiles (dtype in_dt) so that
    normalized = src*Ab + Bb (gamma/beta applied separately)."""
    mm = nc.tensor.matmul
    Alu = mybir.AluOpType
    Act = mybir.ActivationFunctionType
    ps_s = pstat.tile([1, 512], F32, tag="ps_s", name="ps_s")
    ps_q = pstat.tile([1, 512], F32, tag="ps_q", name="ps_q")
    for k in range(len(src_tiles)):
        sq = pool.tile([128, 512], BF, tag="ln_sq", bufs=2, name="ln_sq")
        nc.scalar.activation(out=sq, in_=src_tiles[k], func=Act.Square)
        if in_dt == BF:
            xb = src_tiles[k]
        else:
            xbt = pool.tile([128, 512], BF, tag="ln_xb", bufs=2, name="ln_xb")
            nc.scalar.copy(out=xbt, in_=src_tiles[k])
            xb = xbt
        mm(out=ps_s, lhsT=ones_col, rhs=xb,
           start=(k == 0), stop=(k == KD - 1))
        mm(out=ps_q, lhsT=ones_col, rhs=sq,
           start=(k == 0), stop=(k == KD - 1))
    m_sb = pool.tile([1, 512], F32, tag="ln_m", bufs=1, name="ln_m")
    e2 = pool.tile([1, 512], F32, tag="ln_e2", bufs=1, name="ln_e2")
    nc.vector.tensor_scalar_mul(out=m_sb, in0=ps_s, scalar1=1.0 / D)
    nc.vector.tensor_scalar_mul(out=e2, in0=ps_q, scalar1=1.0 / D)
    msq = pool.tile([1, 512], F32, tag="ln_msq", bufs=1, name="ln_msq")
    nc.vector.tensor_mul(out=msq, in0=m_sb, in1=m_sb)
    var = pool.tile([1, 512], F32, tag="ln_var", bufs=1, name="ln_var")
    nc.vector.scalar_tensor_tensor(
        out=var, in0=e2, scalar=EPS, in1=msq,
        op0=Alu.add, op1=Alu.subtract)
    sd = pool.tile([1, 512], F32, tag="ln_sd", bufs=1, name="ln_sd")
    nc.scalar.activation(out=sd, in_=var, func=Act.Sqrt)
    a_row = pool.tile([1, 512], F32, tag="ln_a", bufs=1, name="ln_a")
    nc.vector.reciprocal(out=a_row, in_=sd)
    b_row = pool.tile([1, 512], F32, tag="ln_b", bufs=1, name="ln_b")
    nc.vector.scalar_tensor_tensor(
        out=b_row, in0=m_sb, scalar=-1.0, in1=a_row,
        op0=Alu.mult, op1=Alu.mult)
    if in_dt == BF:
        ac = pool.tile([1, 512], BF, tag="ln_ac", bufs=1, name="ln_ac")
        bc_ = pool.tile([1, 512], BF, tag="ln_bc", bufs=1, name="ln_bc")
        nc.vector.tensor_copy(out=ac, in_=a_row)
        nc.vector.tensor_copy(out=bc_, in_=b_row)
        a_row, b_row = ac, bc_
    Ab = pool.tile([128, 512], in_dt, tag="ln_Ab", bufs=2, name="ln_Ab")
    Bb = pool.tile([128, 512], in_dt, tag="ln_Bb", bufs=2, name="ln_Bb")
    nc.gpsimd.partition_broadcast(Ab, a_row)
    nc.gpsimd.partition_broadcast(Bb, b_row)
    return Ab, Bb


def _ln_apply(nc, pool, src_k, Ab, Bb, g_sb, be_sb, k, out_slice, in_dt):
    """out = (src*Ab + Bb)*g[k] + be[k], bf16."""
    Alu = mybir.AluOpType
    t1 = pool.tile([128, 512], in_dt, tag="ln_t1", bufs=2, name="ln_t1")
    nc.vector.tensor_mul(out=t1, in0=src_k, in1=Ab)
    nc.vector.tensor_add(out=t1, in0=t1, in1=Bb)
    nc.vector.tensor_scalar(
        out=out_slice, in0=t1,
        scalar1=g_sb[:, k:k + 1], scalar2=be_sb[:, k:k + 1],
        op0=Alu.mult, op1=Alu.add)


def _build():
    nc = bacc.Bacc("TRN2", target_bir_lowering=False, debug=False,
                   num_devices=NCORES)

    tensors = {}
    tensors["xT"] = nc.dram_tensor("xT", [D, T], F32, kind="ExternalInput").ap()
    for name, shape, dt in (
        ("wq", [D, LHE], BF), ("wk", [D, LHE], BF), ("wv", [D, LHE], BF),
        ("wo", [LHE, D], BF), ("w1", [D, LFF], BF), ("w2", [LFF, D], BF),
        ("b1l", [LFF], F32), ("bo", [D], F32), ("b2", [D], F32),
        ("g1", [D], F32), ("be1", [D], F32), ("g2", [D], F32),
        ("be2", [D], F32),
    ):
        tensors[name] = nc.dram_tensor(name, shape, dt,
                                       kind="ExternalInput").ap()
    tensors["outT"] = nc.dram_tensor("out", [D, T], F32,
                                     kind="ExternalOutput").ap()

    with tile.TileContext(nc, num_cores=NCORES) as tc:
        _emit(nc, tc, tensors)

    nc.compile()
    return nc


_NC_CACHE = None


def _get_nc():
    global _NC_CACHE
    if _NC_CACHE is None:
        _NC_CACHE = _build()
    return _NC_CACHE


def _shard_inputs(x, Wq, Wk, Wv, Wo, bo, W1, b1, W2, b2, g1, be1, g2, be2):
    """Build the 8 per-core input maps."""
    bf = lambda a: np.ascontiguousarray(a).astype(BF16NP)
    f32 = lambda a: np.ascontiguousarray(a, dtype=np.float32)

    in_maps = []
    for c in range(NCORES):
        b, half = divmod(c, TP)
        heads = slice(half * LH, (half + 1) * LH)
        ffs = slice(half * LFF, (half + 1) * LFF)
        hes = slice(half * LHE, (half + 1) * LHE)
        # [H, D, HS] slice -> concat local heads along last dim -> [D, LHE]
        wq_l = np.concatenate(list(np.asarray(Wq)[heads]), axis=1)
        wk_l = np.concatenate(list(np.asarray(Wk)[heads]), axis=1)
        wv_l = np.concatenate(list(np.asarray(Wv)[heads]), axis=1)
        in_maps.append({
            "xT": f32(np.asarray(x)[b].T),
            "wq": bf(wq_l), "wk": bf(wk_l), "wv": bf(wv_l),
            "wo": bf(np.asarray(Wo)[hes, :]),
            "w1": bf(np.asarray(W1)[:, ffs]), "w2": bf(np.asarray(W2)[ffs, :]),
            "b1l": f32(np.asarray(b1)[ffs]),
            "bo": f32(bo), "b2": f32(b2),
            "g1": f32(g1), "be1": f32(be1), "g2": f32(g2), "be2": f32(be2),
        })
    return in_maps


def kernel(x, Wq, Wk, Wv, Wo, bo, W1, b1, W2, b2, g1, be1, g2, be2,
           _trace=False):
    nc = _get_nc()
    in_maps = _shard_inputs(x, Wq, Wk, Wv, Wo, bo, W1, b1, W2, b2,
                            g1, be1, g2, be2)
    res = run_bass_kernel_spmd(nc, in_maps, list(range(NCORES)),
                               trace=_trace)
    out = np.empty((B, T, D), dtype=np.float32)
    for b in range(B):
        out[b] = res.results[TP * b]["out"].T
    if _trace:
        kernel.last_exec_time_ns = res.exec_time_ns
        kernel.last_results = res
    return out



# revision 13
# speedup vs baseline: 1.3760x; 1.3760x over previous
"""Trainium2 Bass kernel for a pre-LN transformer block (B=4, T=2048, D=1024,
H=16, HS=64, FF=4096, causal attention).

Sharding: data-parallel over batches x 2-way tensor-parallel attention
(8 heads/core over all T) -> pair ReduceScatter of the attention-output
projection over the sequence dim -> sequence-parallel FFN (full FF width,
T/2 rows per core).  No AllReduce anywhere; each core emits the final
output for its own T/2 rows.

Core c (0..7): batch b = c//2, half = c%2.  half h owns t-slices
[ci*512 + h*256, ci*512 + h*256 + 256) for ci in 0..3.

Layout: activations feature-major (d on partitions, t on free dim).
LayerNorm gains are folded into the weights on the host; LN on-chip is
just (x - mu) * inv_sigma with stats from DVE adder trees + one-column
matmuls that share the projection PSUM slots.  Attention is
phase-separated per (head-pair, chunk): score matmuls run a few steps
ahead of the o-accum matmuls with exp ([128,2,512] double-bank ACT ops)
in between, so the PE never stalls behind the scalar engine.
"""

import numpy as np
import ml_dtypes

import concourse.bacc as bacc
import concourse.bass as bass
import concourse.mybir as mybir
import concourse.tile as tile
from concourse.bass_utils import run_bass_kernel_spmd

BF16NP = ml_dtypes.bfloat16

B, T, D, H, HS, FF = 4, 2048, 1024, 16, 64, 4096
EPS = 1e-5
NCORES = 8
TP = 2
LH = H // TP          # 8 local heads
LHE = LH * HS         # 512 local head-embed width
LT = T // TP          # 1024 local rows (FFN/output)
KD = D // 128         # 8 d k-tiles
KHE = LHE // 128      # 4 he k-tiles
KFF = FF // 128       # 32 ff tiles
NCH = T // 512        # 4 t-chunks of 512
NST = T // 128        # 16 s-tiles of 128
PAIRS = [[0, 1], [2, 3], [4, 5], [6, 7]]
OA_LAG = 2            # psc tiles in flight between scores and o-accum

F32 = mybir.dt.float32
BF = mybir.dt.bfloat16


def _ln_stats(nc, pool, psum_pool, psum_tag, src, ones_col, psum_bufs=2):
    """src: [128, KD, 512] AP.  Returns (Ab, Bb) [128,512] bf16 broadcast
    tiles so that xn = src*Ab + Bb.  Stats via bf16 DVE adder trees plus
    two one-column matmuls that borrow slots from psum_pool/psum_tag."""
    mm = nc.tensor.matmul
    Alu = mybir.AluOpType
    Act = mybir.ActivationFunctionType

    def lvl(tg, n):
        return pool.tile([128, 512], BF, tag=f"{tg}{n}", bufs=2, name=tg)

    s2, q2 = [], []
    for i in range(4):
        s = lvl("lts", 2)
        nc.vector.tensor_add(out=s, in0=src[:, 2 * i, :],
                             in1=src[:, 2 * i + 1, :])
        s2.append(s)
        sqa = pool.tile([128, 512], BF, tag="ln_sq", bufs=2, name="ln_sq")
        nc.vector.tensor_mul(out=sqa, in0=src[:, 2 * i, :],
                             in1=src[:, 2 * i, :])
        sqb = pool.tile([128, 512], BF, tag="ln_sq", bufs=2, name="ln_sq")
        nc.vector.tensor_mul(out=sqb, in0=src[:, 2 * i + 1, :],
                             in1=src[:, 2 * i + 1, :])
        q = lvl("ltq", 2)
        nc.vector.tensor_add(out=q, in0=sqa, in1=sqb)
        q2.append(q)
    s4, q4_ = [], []
    for i in range(2):
        s = lvl("lts", 4)
        nc.vector.tensor_add(out=s, in0=s2[2 * i], in1=s2[2 * i + 1])
        s4.append(s)
        q = lvl("ltq", 4)
        nc.vector.tensor_add(out=q, in0=q2[2 * i], in1=q2[2 * i + 1])
        q4_.append(q)
    s_all = lvl("lts", 8)
    nc.vector.tensor_add(out=s_all, in0=s4[0], in1=s4[1])
    q_all = lvl("ltq", 8)
    nc.vector.tensor_add(out=q_all, in0=q4_[0], in1=q4_[1])

    ps_s = psum_pool.tile([1, 512], F32, tag=psum_tag, bufs=psum_bufs,
                          name="ps_s")
    ps_q = psum_pool.tile([1, 512], F32, tag=psum_tag, bufs=psum_bufs,
                          name="ps_q")
    mm(out=ps_s, lhsT=ones_col, rhs=s_all, start=True, stop=True)
    mm(out=ps_q, lhsT=ones_col, rhs=q_all, start=True, stop=True)

    m = pool.tile([1, 512], F32, tag="ln_m", bufs=1, name="ln_m")
    e2 = pool.tile([1, 512], F32, tag="ln_e2", bufs=1, name="ln_e2")
    nc.vector.tensor_scalar_mul(out=m, in0=ps_s, scalar1=1.0 / D)
    nc.vector.tensor_scalar_mul(out=e2, in0=ps_q, scalar1=1.0 / D)
    msq = pool.tile([1, 512], F32, tag="ln_msq", bufs=1, name="ln_msq")
    nc.vector.tensor_mul(out=msq, in0=m, in1=m)
    var = pool.tile([1, 512], F32, tag="ln_var", bufs=1, name="ln_var")
    nc.vector.scalar_tensor_tensor(out=var, in0=e2, scalar=EPS, in1=msq,
                                   op0=Alu.add, op1=Alu.subtract)
    sd = pool.tile([1, 512], F32, tag="ln_sd", bufs=1, name="ln_sd")
    nc.scalar.activation(out=sd, in_=var, func=Act.Sqrt)
    a_row = pool.tile([1, 512], F32, tag="ln_a", bufs=1, name="ln_a")
    nc.vector.reciprocal_approx_fast(out=a_row, in_=sd)
    b_row = pool.tile([1, 512], F32, tag="ln_b", bufs=1, name="ln_b")
    nc.vector.scalar_tensor_tensor(out=b_row, in0=m, scalar=-1.0, in1=a_row,
                                   op0=Alu.mult, op1=Alu.mult)
    ac = pool.tile([1, 512], BF, tag="ln_ac", bufs=1, name="ln_ac")
    bc = pool.tile([1, 512], BF, tag="ln_bc", bufs=1, name="ln_bc")
    nc.vector.tensor_copy(out=ac, in_=a_row)
    nc.vector.tensor_copy(out=bc, in_=b_row)
    Ab = pool.tile([128, 512], BF, tag="ln_Ab", bufs=2, name="ln_Ab")
    Bb = pool.tile([128, 512], BF, tag="ln_Bb", bufs=2, name="ln_Bb")
    nc.gpsimd.partition_broadcast(Ab, ac)
    nc.gpsimd.partition_broadcast(Bb, bc)
    return Ab, Bb


def _ln_apply(nc, pool, src_k, Ab, Bb, out_slice):
    """out = src*Ab + Bb (bf16)."""
    t1 = pool.tile([128, 512], BF, tag="ln_t1", bufs=2, name="ln_t1")
    nc.vector.tensor_mul(out=t1, in0=src_k, in1=Ab)
    nc.vector.tensor_add(out=out_slice, in0=t1, in1=Bb)


def _emit(nc, tc, t):
    mm = nc.tensor.matmul
    Alu = mybir.AluOpType
    Act = mybir.ActivationFunctionType

    xT_v = t["xT"].rearrange("(k p) t -> p k t", p=128)
    xres_v = t["xresT"].rearrange("(k p) t -> p k t", p=128)
    w1_v = t["w1"].rearrange("(k p) e -> p k e", p=128)
    w2_v = t["w2"].rearrange("(k p) e -> p k e", p=128)
    outT_v = t["outT"]

    # ---------------- persistent pools ----------------
    dram = tc.alloc_tile_pool(name="dram", bufs=1, space="DRAM")
    rs_in = [dram.tile([TP, D, 256], BF, name=f"rsi{c}") for c in range(NCH)]
    rs_out = [dram.tile([D, 256], BF, name=f"rso{c}") for c in range(NCH)]

    consts = tc.alloc_tile_pool(name="consts", bufs=1)
    ones_col = consts.tile([128, 1], BF)
    nc.vector.memset(ones_col, 1.0)

    bq_sb = consts.tile([128, KHE], F32)
    bk_sb = consts.tile([128, KHE], F32)
    bo2_sb = consts.tile([128, KD], F32)
    b2_sb = consts.tile([128, KD], F32)
    b1_sb = consts.tile([128, KFF], F32)
    for name, dst in (("bq", bq_sb), ("bk", bk_sb),
                      ("bo2", bo2_sb), ("b2", b2_sb)):
        nc.sync.dma_start(out=dst, in_=t[name].rearrange("(k p) -> p k", p=128))
    nc.sync.dma_start(out=b1_sb, in_=t["b1f"].rearrange("(k p) -> p k", p=128))
    # v bias broadcast over all partitions: [128, LHE]
    bvb = consts.tile([128, LHE], BF)
    bv_row = consts.tile([1, LHE], BF)
    nc.gpsimd.dma_start(out=bv_row,
                        in_=t["bv"].rearrange("(o e) -> o e", o=1))
    nc.gpsimd.partition_broadcast(bvb, bv_row)

    # causal masks for the 4 diagonal offsets
    masks = []
    for midx in range(4):
        mk = consts.tile([128, 512], BF, name=f"mask{midx}")
        nc.vector.memset(mk, 1.0)
        nc.gpsimd.affine_select(
            out=mk, in_=mk, compare_op=Alu.is_ge, fill=0.0,
            base=-(midx * 128), channel_multiplier=-1, pattern=[[1, 512]])
        masks.append(mk)

    wlate = tc.alloc_tile_pool(name="wlate", bufs=1)
    wo_sb = wlate.tile([128, KHE, D], BF, tag="wo")
    nc.sync.dma_start(out=wo_sb, in_=t["wo"].rearrange("(k p) e -> p k e", p=128))

    # ================= Stage A: LN1 + QKV + attention + Wo + RS ============
    with tc.tile_pool(name="abc", bufs=1) as ab:
        kT = ab.tile([128, LH // 2, T], BF, tag="kT")
        qT = ab.tile([128, LH // 2, T], BF, tag="qT")
        vS = ab.tile([128, NST, LH * 65], BF, tag="vS")
        nc.vector.memset(vS, 1.0)
        wq_sb = ab.tile([128, KD, LHE], BF, tag="wq")
        wk_sb = ab.tile([128, KD, LHE], BF, tag="wk")
        wv_sb = ab.tile([128, KD, LHE], BF, tag="wv")
        for src, dst in ((t["wq"], wq_sb), (t["wk"], wk_sb), (t["wv"], wv_sb)):
            nc.sync.dma_start(out=dst,
                              in_=src.rearrange("(k p) e -> p k e", p=128))

        with tc.tile_pool(name="projpsum", bufs=2, space="PSUM") as pproj, \
             tc.tile_pool(name="scpsum", bufs=OA_LAG, space="PSUM") as psc, \
             tc.tile_pool(name="oapsum", bufs=2, space="PSUM") as poa:

            def ln1(ci):
                xf = ab.tile([128, KD, 512], F32, tag="xf", bufs=2, name="xf")
                nc.sync.dma_start(out=xf,
                                  in_=xT_v[:, :, ci * 512:ci * 512 + 512])
                Ab, Bb = _ln_stats(nc, ab, pproj, "ps_proj", xf, ones_col)
                hT = ab.tile([128, KD, 512], BF, tag="hT", bufs=2, name="hT")
                for k in range(KD):
                    _ln_apply(nc, ab, xf[:, k, :], Ab, Bb, hT[:, k, :])
                return hT

            def proj(ci, hT):
                c0 = ci * 512
                for w_sb, dst, bias in ((wk_sb, kT, bk_sb), (wq_sb, qT, bq_sb)):
                    for et in range(LH // 2):
                        ps = pproj.tile([128, 512], F32, tag="ps_proj", bufs=2,
                                        name="ps_proj")
                        for k in range(KD):
                            mm(out=ps, lhsT=w_sb[:, k, et * 128:(et + 1) * 128],
                               rhs=hT[:, k, :],
                               start=(k == 0), stop=(k == KD - 1))
                        nc.vector.tensor_scalar(
                            out=dst[:, et, c0:c0 + 512], in0=ps,
                            scalar1=bias[:, et:et + 1], scalar2=None,
                            op0=Alu.add)
                for sti in range(4):
                    st = ci * 4 + sti
                    ps = pproj.tile([128, LHE], F32, tag="ps_proj", bufs=2,
                                    name="ps_v")
                    for k in range(KD):
                        mm(out=ps, lhsT=hT[:, k, sti * 128:sti * 128 + 128],
                           rhs=wv_sb[:, k, :],
                           start=(k == 0), stop=(k == KD - 1))
                    nc.vector.tensor_add(
                        out=vS[:, st, :].rearrange("p (h e) -> p h e",
                                                   h=LH)[:, :, 0:64],
                        in0=ps.rearrange("p (h e) -> p h e", e=64),
                        in1=bvb.rearrange("p (h e) -> p h e", e=64))

            def attention(ci, oT):
                c0 = ci * 512
                nb = 4 * (ci + 1)
                den8 = ab.tile([LH, 512], F32, tag="den8", bufs=1, name="den8")
                ous = {}
                for hp in range(LH // 2):
                    po = [poa.tile([65, 512], F32, tag="po", bufs=2, name="po")
                          for _ in range(2)]
                    exs = [None] * nb

                    def scores(sb):
                        s0 = sb * 128
                        ps2 = psc.tile([128, 2, 512], F32, tag="ps_sc",
                                       bufs=OA_LAG, name="ps_sc")
                        for hi in range(2):
                            mm(out=ps2[:, hi, :],
                               lhsT=kT[hi * 64:hi * 64 + 64, hp, s0:s0 + 128],
                               rhs=qT[hi * 64:hi * 64 + 64, hp, c0:c0 + 512],
                               start=True, stop=True)
                        ex = ab.tile([128, 2, 512], BF, tag="ex",
                                     bufs=OA_LAG + 1, name="ex")
                        nc.scalar.activation(out=ex, in_=ps2, func=Act.Exp)
                        midx = sb - 4 * ci
                        if midx >= 0:
                            for hi in range(2):
                                nc.vector.tensor_mul(out=ex[:, hi, :],
                                                     in0=ex[:, hi, :],
                                                     in1=masks[midx])
                        exs[sb] = ex

                    def oacc(sb):
                        for hi in range(2):
                            h_loc = hp * 2 + hi
                            mm(out=po[hi],
                               lhsT=vS[:, sb, h_loc * 65:h_loc * 65 + 65],
                               rhs=exs[sb][:, hi, :],
                               start=(sb == 0), stop=(sb == nb - 1))

                    for step in range(nb + OA_LAG):
                        if step < nb:
                            scores(step)
                        if step >= OA_LAG:
                            oacc(step - OA_LAG)

                    for hi in range(2):
                        h_loc = hp * 2 + hi
                        ou = ab.tile([64, 512], BF, tag="ou", bufs=8, name="ou")
                        nc.vector.tensor_copy(out=ou, in_=po[hi][0:64, :])
                        ous[h_loc] = ou
                        dnr = ab.tile([1, 512], F32, tag="dnr", bufs=2,
                                      name="dnr")
                        nc.vector.tensor_copy(out=dnr, in_=po[hi][64:65, :])
                        nc.scalar.dma_start(out=den8[h_loc:h_loc + 1, :],
                                            in_=dnr)
                rec8 = ab.tile([LH, 512], F32, tag="rec8", bufs=1, name="rec8")
                nc.vector.reciprocal_approx_fast(out=rec8, in_=den8)
                rb8 = ab.tile([LH, 512], BF, tag="rb8", bufs=1, name="rb8")
                nc.vector.tensor_copy(out=rb8, in_=rec8)
                for h_loc in range(LH):
                    rbt = ab.tile([1, 512], BF, tag="rbt", bufs=2, name="rbt")
                    nc.scalar.dma_start(out=rbt, in_=rb8[h_loc:h_loc + 1, :])
                    bc = ab.tile([64, 512], BF, tag="bc", bufs=2, name="bc")
                    nc.gpsimd.partition_broadcast(bc, rbt)
                    nc.vector.tensor_mul(
                        out=oT[(h_loc % 2) * 64:(h_loc % 2) * 64 + 64,
                               h_loc // 2, :],
                        in0=ous[h_loc], in1=bc)

            def wo_rs(ci, oT):
                rsv = rs_in[ci].rearrange("j (k p) t -> j k p t", p=128)
                for dt in range(KD):
                    ps = pproj.tile([128, 512], F32, tag="ps_proj",
                                    bufs=2, name="ps_wo")
                    for k in range(KHE):
                        mm(out=ps, lhsT=wo_sb[:, k, dt * 128:(dt + 1) * 128],
                           rhs=oT[:, k, :],
                           start=(k == 0), stop=(k == KHE - 1))
                    stg = ab.tile([128, 512], BF, tag="stg1", bufs=2,
                                  name="stg1")
                    nc.vector.tensor_scalar(
                        out=stg, in0=ps, scalar1=bo2_sb[:, dt:dt + 1],
                        scalar2=None, op0=Alu.add)
                    for j in range(TP):
                        nc.sync.dma_start(
                            out=rsv[j, dt, :, :],
                            in_=stg[:, j * 256:(j + 1) * 256])
                nc.gpsimd.collective_compute(
                    "ReduceScatter", Alu.add, replica_groups=PAIRS,
                    ins=[rs_in[ci].opt()], outs=[rs_out[ci].opt()])

            hT = ln1(0)
            for ci in range(NCH):
                proj(ci, hT)
                if ci + 1 < NCH:
                    hT = ln1(ci + 1)
                oT = ab.tile([128, KHE, 512], BF, tag="oT", bufs=2, name="oT")
                attention(ci, oT)
                wo_rs(ci, oT)

    # ================= Stage B: residual + LN2 + FFN (own T/2 rows) ========
    with tc.tile_pool(name="de", bufs=1) as de, \
         tc.tile_pool(name="upsum", bufs=3, space="PSUM") as pu, \
         tc.tile_pool(name="fpsum", bufs=2, space="PSUM") as pf:

        w2_sb = de.tile([128, KFF, D], BF, tag="w2t", bufs=1, name="w2t")
        for q4 in range(4):
            nc.sync.dma_start(out=w2_sb[:, q4 * 8:(q4 + 1) * 8, :],
                              in_=w2_v[:, q4 * 8:(q4 + 1) * 8, :])

        for lc in range(2):
            c0 = lc * 512
            # residual: xmid = xres + rs_out  (bf16 residual stream)
            xr = de.tile([128, KD, 512], BF, tag="xr", bufs=1, name="xr")
            nc.gpsimd.dma_start(out=xr, in_=xres_v[:, :, c0:c0 + 512])
            arr = de.tile([128, KD, 2, 256], BF, tag="arr", bufs=1, name="arr")
            for j in range(2):
                nc.sync.dma_start(
                    out=arr[:, :, j, :],
                    in_=rs_out[2 * lc + j].rearrange("(k p) t -> p k t", p=128))
            xmid = de.tile([128, KD, 512], BF, tag="xmid", bufs=2,
                           name="xmid")
            for k in range(KD):
                nc.vector.tensor_add(
                    out=xmid[:, k, :], in0=xr[:, k, :],
                    in1=arr[:, k, :, :].rearrange("p j t -> p (j t)"))

            # LN2 (gains folded into W1/b1f on host)
            Ab2, Bb2 = _ln_stats(nc, de, pu, "ps_u", xmid, ones_col, psum_bufs=3)
            h2 = de.tile([128, KD, 512], BF, tag="h2", bufs=1, name="h2")
            for k in range(KD):
                _ln_apply(nc, de, xmid[:, k, :], Ab2, Bb2, h2[:, k, :])

            # FFN up: u = relu(h2 @ W1 + b1f)   (relu+bias on DVE)
            u = de.tile([128, KFF, 512], BF, tag="u", bufs=1, name="u")
            for q16 in range(16):
                w1t = de.tile([128, KD, 256], BF, tag="w1t", bufs=2,
                              name="w1t")
                nc.sync.dma_start(out=w1t,
                                  in_=w1_v[:, :, q16 * 256:(q16 + 1) * 256])
                for fi in range(2):
                    fft = q16 * 2 + fi
                    ps = pu.tile([128, 512], F32, tag="ps_u", bufs=3,
                                 name="ps_u")
                    for k in range(KD):
                        mm(out=ps,
                           lhsT=w1t[:, k, fi * 128:fi * 128 + 128],
                           rhs=h2[:, k, :],
                           start=(k == 0), stop=(k == KD - 1))
                    nc.vector.tensor_scalar(
                        out=u[:, fft, :], in0=ps,
                        scalar1=b1_sb[:, fft:fft + 1], scalar2=0.0,
                        op0=Alu.add, op1=Alu.max)

            # FFN down + bias + residual -> store
            for dt in range(KD):
                ps = pf.tile([128, 512], F32, tag="ps_f", bufs=2, name="ps_f")
                for k2 in range(KFF):
                    mm(out=ps,
                       lhsT=w2_sb[:, k2, dt * 128:(dt + 1) * 128],
                       rhs=u[:, k2, :],
                       start=(k2 == 0), stop=(k2 == KFF - 1))
                o_f = de.tile([128, 512], F32, tag="o_f", bufs=2, name="o_f")
                nc.vector.scalar_tensor_tensor(
                    out=o_f, in0=ps, scalar=b2_sb[:, dt:dt + 1],
                    in1=xmid[:, dt, :], op0=Alu.add, op1=Alu.add)
                nc.sync.dma_start(
                    out=outT_v[dt * 128:(dt + 1) * 128, c0:c0 + 512],
                    in_=o_f)

    wlate.release()
    consts.release()
    dram.release()


def _build():
    nc = bacc.Bacc("TRN2", target_bir_lowering=False, debug=False,
                   num_devices=NCORES)

    tensors = {}
    tensors["xT"] = nc.dram_tensor("xT", [D, T], F32, kind="ExternalInput").ap()
    tensors["xresT"] = nc.dram_tensor("xresT", [D, LT], F32,
                                      kind="ExternalInput").ap()
    for name, shape, dt in (
        ("wq", [D, LHE], BF), ("wk", [D, LHE], BF), ("wv", [D, LHE], BF),
        ("wo", [LHE, D], BF), ("w1", [D, FF], BF), ("w2", [FF, D], BF),
        ("bq", [LHE], F32), ("bk", [LHE], F32), ("bv", [LHE], F32),
        ("b1f", [FF], F32), ("bo2", [D], F32), ("b2", [D], F32),
    ):
        tensors[name] = nc.dram_tensor(name, shape, dt,
                                       kind="ExternalInput").ap()
    tensors["outT"] = nc.dram_tensor("out", [D, LT], F32,
                                     kind="ExternalOutput").ap()

    with tile.TileContext(nc, num_cores=NCORES) as tc:
        _emit(nc, tc, tensors)

    nc.compile()
    return nc


_NC_CACHE = None


def _get_nc():
    global _NC_CACHE
    if _NC_CACHE is None:
        _NC_CACHE = _build()
    return _NC_CACHE


def _shard_inputs(x, Wq, Wk, Wv, Wo, bo, W1, b1, W2, b2, g1, be1, g2, be2):
    """Build the 8 per-core input maps (LN gains folded into weights)."""
    bf = lambda a: np.ascontiguousarray(a).astype(BF16NP)
    f32 = lambda a: np.ascontiguousarray(a, dtype=np.float32)

    x = np.asarray(x, dtype=np.float32)
    Wq = np.asarray(Wq, dtype=np.float32)
    Wk = np.asarray(Wk, dtype=np.float32)
    Wv = np.asarray(Wv, dtype=np.float32)
    Wo = np.asarray(Wo, dtype=np.float32)
    W1 = np.asarray(W1, dtype=np.float32)
    W2 = np.asarray(W2, dtype=np.float32)
    g1 = np.asarray(g1, dtype=np.float32)
    be1 = np.asarray(be1, dtype=np.float32)
    g2 = np.asarray(g2, dtype=np.float32)
    be2 = np.asarray(be2, dtype=np.float32)
    b1 = np.asarray(b1, dtype=np.float32)

    scale = float(HS) ** -0.5
    # fold g1 into QKV weights, be1 into QKV biases; fold the score scale
    # into Wq/bq.  Per-head [H, D, HS] -> concat heads -> [D, H*HS].
    wq_f = (g1[None, :, None] * Wq).transpose(1, 0, 2).reshape(D, D) * scale
    wk_f = (g1[None, :, None] * Wk).transpose(1, 0, 2).reshape(D, D)
    wv_f = (g1[None, :, None] * Wv).transpose(1, 0, 2).reshape(D, D)
    bq_f = np.einsum("d,hde->he", be1, Wq).reshape(D) * scale
    bk_f = np.einsum("d,hde->he", be1, Wk).reshape(D)
    bv_f = np.einsum("d,hde->he", be1, Wv).reshape(D)
    # fold g2/be2 into W1/b1
    w1_f = g2[:, None] * W1
    b1_f = b1 + be2 @ W1

    in_maps = []
    for c in range(NCORES):
        b, half = divmod(c, TP)
        hes = slice(half * LHE, (half + 1) * LHE)
        xt = x[b].T
        xres = np.concatenate(
            [xt[:, ci * 512 + half * 256: ci * 512 + half * 256 + 256]
             for ci in range(NCH)], axis=1)
        in_maps.append({
            "xT": f32(xt),
            "xresT": f32(xres),
            "wq": bf(wq_f[:, hes]), "wk": bf(wk_f[:, hes]),
            "wv": bf(wv_f[:, hes]),
            "bq": f32(bq_f[hes]), "bk": f32(bk_f[hes]), "bv": f32(bv_f[hes]),
            "wo": bf(Wo[hes, :]),
            "bo2": f32(np.asarray(bo, dtype=np.float32) / TP),
            "w1": bf(w1_f), "b1f": f32(b1_f),
            "w2": bf(W2), "b2": f32(np.asarray(b2, dtype=np.float32)),
        })
    return in_maps


def kernel(x, Wq, Wk, Wv, Wo, bo, W1, b1, W2, b2, g1, be1, g2, be2,
           _trace=False):
    nc = _get_nc()
    in_maps = _shard_inputs(x, Wq, Wk, Wv, Wo, bo, W1, b1, W2, b2,
                            g1, be1, g2, be2)
    res = run_bass_kernel_spmd(nc, in_maps, list(range(NCORES)),
                               trace=_trace)
    out = np.empty((B, T, D), dtype=np.float32)
    for b in range(B):
        for half in range(TP):
            o = res.results[TP * b + half]["out"]  # [D, LT]
            for ci in range(NCH):
                t0 = ci * 512 + half * 256
                out[b, t0:t0 + 256, :] = o[:, ci * 256:(ci + 1) * 256].T
    if _trace:
        kernel.last_exec_time_ns = res.exec_time_ns
        kernel.last_results = res
    return out


# revision 17
# speedup vs baseline: 1.8095x; 1.3151x over previous
"""Trainium2 Bass kernel for a pre-LN transformer block (B=4, T=2048, D=1024,
H=16, HS=64, FF=4096, causal attention).

Sharding: data-parallel over batches x 2-way tensor-parallel attention
(8 heads/core over all T) -> pair ReduceScatter of the attention-output
projection over the sequence dim -> sequence-parallel FFN (full FF width,
T/2 rows per core).  No AllReduce anywhere; each core emits the final
output for its own T/2 rows.

Core c (0..7): batch b = c//2, half = c%2.  half h owns t-slices
[ci*512 + h*256, ci*512 + h*256 + 256) for ci in 0..3.

Layout: activations feature-major (d on partitions, t on free dim).
LayerNorm gains are folded into the weights on the host; LN on-chip is
just (x - mu) * inv_sigma with stats from DVE adder trees + one-column
matmuls that share the projection PSUM slots.  Attention is
phase-separated per (head-pair, chunk): score matmuls run a few steps
ahead of the o-accum matmuls with exp ([128,2,512] double-bank ACT ops)
in between, so the PE never stalls behind the scalar engine.
"""

import numpy as np
import ml_dtypes

import concourse.bacc as bacc
import concourse.bass as bass
import concourse.mybir as mybir
import concourse.tile as tile
from concourse.bass_utils import run_bass_kernel_spmd

BF16NP = ml_dtypes.bfloat16

B, T, D, H, HS, FF = 4, 2048, 1024, 16, 64, 4096
EPS = 1e-5
NCORES = 8
TP = 2
LH = H // TP          # 8 local heads
LHE = LH * HS         # 512 local head-embed width
LT = T // TP          # 1024 local rows (FFN/output)
KD = D // 128         # 8 d k-tiles
KHE = LHE // 128      # 4 he k-tiles
KFF = FF // 128       # 32 ff tiles
NCH = T // 512        # 4 t-chunks of 512
NST = T // 128        # 16 s-tiles of 128
PAIRS = [[0, 1], [2, 3], [4, 5], [6, 7]]
OA_LAG = 2            # psc tiles in flight between scores and o-accum

F32 = mybir.dt.float32
BF = mybir.dt.bfloat16


def _ln_stats(nc, pool, psum_pool, psum_tag, src, ones_col, psum_bufs=2):
    """src: [128, KD, 512] AP.  Returns (Ab, Bb) [128,512] bf16 broadcast
    tiles so that xn = src*Ab + Bb.  Stats via bf16 DVE adder trees plus
    two one-column matmuls that borrow slots from psum_pool/psum_tag."""
    mm = nc.tensor.matmul
    Alu = mybir.AluOpType
    Act = mybir.ActivationFunctionType

    def lvl(tg, n):
        return pool.tile([128, 512], BF, tag=f"{tg}{n}", bufs=2, name=tg)

    s2, q2 = [], []
    for i in range(4):
        s = lvl("lts", 2)
        nc.vector.tensor_add(out=s, in0=src[:, 2 * i, :],
                             in1=src[:, 2 * i + 1, :])
        s2.append(s)
        sqa = pool.tile([128, 512], BF, tag="ln_sq", bufs=2, name="ln_sq")
        nc.vector.tensor_mul(out=sqa, in0=src[:, 2 * i, :],
                             in1=src[:, 2 * i, :])
        sqb = pool.tile([128, 512], BF, tag="ln_sq", bufs=2, name="ln_sq")
        nc.vector.tensor_mul(out=sqb, in0=src[:, 2 * i + 1, :],
                             in1=src[:, 2 * i + 1, :])
        q = lvl("ltq", 2)
        nc.vector.tensor_add(out=q, in0=sqa, in1=sqb)
        q2.append(q)
    s4, q4_ = [], []
    for i in range(2):
        s = lvl("lts", 4)
        nc.vector.tensor_add(out=s, in0=s2[2 * i], in1=s2[2 * i + 1])
        s4.append(s)
        q = lvl("ltq", 4)
        nc.vector.tensor_add(out=q, in0=q2[2 * i], in1=q2[2 * i + 1])
        q4_.append(q)
    s_all = lvl("lts", 8)
    nc.vector.tensor_add(out=s_all, in0=s4[0], in1=s4[1])
    q_all = lvl("ltq", 8)
    nc.vector.tensor_add(out=q_all, in0=q4_[0], in1=q4_[1])

    ps_s = psum_pool.tile([1, 512], F32, tag=psum_tag, bufs=psum_bufs,
                          name="ps_s")
    ps_q = psum_pool.tile([1, 512], F32, tag=psum_tag, bufs=psum_bufs,
                          name="ps_q")
    mm(out=ps_s, lhsT=ones_col, rhs=s_all, start=True, stop=True)
    mm(out=ps_q, lhsT=ones_col, rhs=q_all, start=True, stop=True)

    m = pool.tile([1, 512], F32, tag="ln_m", bufs=1, name="ln_m")
    e2 = pool.tile([1, 512], F32, tag="ln_e2", bufs=1, name="ln_e2")
    nc.vector.tensor_scalar_mul(out=m, in0=ps_s, scalar1=1.0 / D)
    nc.vector.tensor_scalar_mul(out=e2, in0=ps_q, scalar1=1.0 / D)
    msq = pool.tile([1, 512], F32, tag="ln_msq", bufs=1, name="ln_msq")
    nc.vector.tensor_mul(out=msq, in0=m, in1=m)
    var = pool.tile([1, 512], F32, tag="ln_var", bufs=1, name="ln_var")
    nc.vector.scalar_tensor_tensor(out=var, in0=e2, scalar=EPS, in1=msq,
                                   op0=Alu.add, op1=Alu.subtract)
    sd = pool.tile([1, 512], F32, tag="ln_sd", bufs=1, name="ln_sd")
    nc.scalar.activation(out=sd, in_=var, func=Act.Sqrt)
    a_row = pool.tile([1, 512], F32, tag="ln_a", bufs=1, name="ln_a")
    nc.vector.reciprocal_approx_fast(out=a_row, in_=sd)
    b_row = pool.tile([1, 512], F32, tag="ln_b", bufs=1, name="ln_b")
    nc.vector.scalar_tensor_tensor(out=b_row, in0=m, scalar=-1.0, in1=a_row,
                                   op0=Alu.mult, op1=Alu.mult)
    ac = pool.tile([1, 512], BF, tag="ln_ac", bufs=1, name="ln_ac")
    bc = pool.tile([1, 512], BF, tag="ln_bc", bufs=1, name="ln_bc")
    nc.vector.tensor_copy(out=ac, in_=a_row)
    nc.vector.tensor_copy(out=bc, in_=b_row)
    Ab = pool.tile([128, 512], BF, tag="ln_Ab", bufs=2, name="ln_Ab")
    Bb = pool.tile([128, 512], BF, tag="ln_Bb", bufs=2, name="ln_Bb")
    nc.gpsimd.partition_broadcast(Ab, ac)
    nc.gpsimd.partition_broadcast(Bb, bc)
    return Ab, Bb


def _ln_apply(nc, pool, src_k, Ab, Bb, out_slice):
    """out = src*Ab + Bb (bf16)."""
    t1 = pool.tile([128, 512], BF, tag="ln_t1", bufs=2, name="ln_t1")
    nc.vector.tensor_mul(out=t1, in0=src_k, in1=Ab)
    nc.vector.tensor_add(out=out_slice, in0=t1, in1=Bb)


def _emit(nc, tc, t):
    mm = nc.tensor.matmul
    Alu = mybir.AluOpType
    Act = mybir.ActivationFunctionType

    xT_v = t["xT"].rearrange("(k p) t -> p k t", p=128)
    xres_v = t["xresT"].rearrange("(k p) t -> p k t", p=128)
    w1_v = t["w1"].rearrange("(k p) e -> p k e", p=128)
    w2_v = t["w2"].rearrange("(k p) e -> p k e", p=128)
    outT_v = t["outT"]

    # ---------------- persistent pools ----------------
    dram = tc.alloc_tile_pool(name="dram", bufs=1, space="DRAM")
    rs_in = [dram.tile([TP, D, 256], BF, name=f"rsi{c}") for c in range(NCH)]
    rs_out = [dram.tile([D, 256], BF, name=f"rso{c}") for c in range(NCH)]

    consts = tc.alloc_tile_pool(name="consts", bufs=1)
    ones_col = consts.tile([128, 1], BF)
    nc.vector.memset(ones_col, 1.0)

    bq_sb = consts.tile([128, KHE], F32)
    bk_sb = consts.tile([128, KHE], F32)
    bo2_sb = consts.tile([128, KD], F32)
    b2_sb = consts.tile([128, KD], F32)
    b1_sb = consts.tile([128, KFF], F32)
    for name, dst in (("bq", bq_sb), ("bk", bk_sb),
                      ("bo2", bo2_sb), ("b2", b2_sb)):
        nc.sync.dma_start(out=dst, in_=t[name].rearrange("(k p) -> p k", p=128))
    nc.sync.dma_start(out=b1_sb, in_=t["b1f"].rearrange("(k p) -> p k", p=128))
    # v bias broadcast over all partitions: [128, LHE]
    bvb = consts.tile([128, LHE], BF)
    bv_row = consts.tile([1, LHE], BF)
    nc.gpsimd.dma_start(out=bv_row,
                        in_=t["bv"].rearrange("(o e) -> o e", o=1))
    nc.gpsimd.partition_broadcast(bvb, bv_row)

    wlate = tc.alloc_tile_pool(name="wlate", bufs=1)
    wo_sb = wlate.tile([128, KHE, D], BF, tag="wo")
    nc.sync.dma_start(out=wo_sb, in_=t["wo"].rearrange("(k p) e -> p k e", p=128))

    # ================= Stage A: LN1 + QKV + attention + Wo + RS ============
    with tc.tile_pool(name="abc", bufs=1) as ab:
        kT = ab.tile([128, LH // 2, T], BF, tag="kT")
        qT = ab.tile([128, LH // 2, T], BF, tag="qT")
        vS = ab.tile([128, NST, LH * 65], BF, tag="vS")
        nc.vector.memset(vS, 1.0)
        wq_sb = ab.tile([128, KD, LHE], BF, tag="wq")
        wk_sb = ab.tile([128, KD, LHE], BF, tag="wk")
        wv_sb = ab.tile([128, KD, LHE], BF, tag="wv")
        for src, dst in ((t["wq"], wq_sb), (t["wk"], wk_sb), (t["wv"], wv_sb)):
            nc.sync.dma_start(out=dst,
                              in_=src.rearrange("(k p) e -> p k e", p=128))

        with tc.tile_pool(name="projpsum", bufs=2, space="PSUM") as pproj, \
             tc.tile_pool(name="scpsum", bufs=OA_LAG, space="PSUM") as psc, \
             tc.tile_pool(name="oapsum", bufs=2, space="PSUM") as poa:

            def ln1(ci):
                xf = ab.tile([128, KD, 512], F32, tag="xf", bufs=2, name="xf")
                nc.sync.dma_start(out=xf,
                                  in_=xT_v[:, :, ci * 512:ci * 512 + 512])
                Ab, Bb = _ln_stats(nc, ab, pproj, "ps_proj", xf, ones_col)
                hT = ab.tile([128, KD, 512], BF, tag="hT", bufs=2, name="hT")
                for k in range(KD):
                    _ln_apply(nc, ab, xf[:, k, :], Ab, Bb, hT[:, k, :])
                return hT

            def proj_units(ci, hT):
                c0 = ci * 512
                units = []
                for w_sb, dst, bias in ((wk_sb, kT, bk_sb), (wq_sb, qT, bq_sb)):
                    for et in range(LH // 2):
                        def u(w_sb=w_sb, dst=dst, bias=bias, et=et, hT=hT,
                              c0=c0):
                            ps = pproj.tile([128, 512], F32, tag="ps_proj",
                                            bufs=2, name="ps_proj")
                            for k in range(KD):
                                mm(out=ps,
                                   lhsT=w_sb[:, k, et * 128:(et + 1) * 128],
                                   rhs=hT[:, k, :],
                                   start=(k == 0), stop=(k == KD - 1))
                            nc.vector.tensor_scalar(
                                out=dst[:, et, c0:c0 + 512], in0=ps,
                                scalar1=bias[:, et:et + 1], scalar2=None,
                                op0=Alu.add)
                        units.append(u)
                for sti in range(4):
                    st = ci * 4 + sti
                    def u(sti=sti, st=st, hT=hT):
                        ps = pproj.tile([128, LHE], F32, tag="ps_proj", bufs=2,
                                        name="ps_v")
                        for k in range(KD):
                            mm(out=ps,
                               lhsT=hT[:, k, sti * 128:sti * 128 + 128],
                               rhs=wv_sb[:, k, :],
                               start=(k == 0), stop=(k == KD - 1))
                        nc.vector.tensor_add(
                            out=vS[:, st, :].rearrange("p (h e) -> p h e",
                                                       h=LH)[:, :, 0:64],
                            in0=ps.rearrange("p (h e) -> p h e", e=64),
                            in1=bvb.rearrange("p (h e) -> p h e", e=64))
                    units.append(u)
                return units

            def attention(ci, oT, pending):
                c0 = ci * 512
                nb = 4 * (ci + 1)
                total_steps = (LH // 2) * (nb + OA_LAG)
                spacing = (max(1, total_steps // len(pending))
                           if pending else 0)
                stepctr = 0
                for hp in range(LH // 2):
                    po = [poa.tile([65, 512], F32, tag="po", bufs=2, name="po")
                          for _ in range(2)]
                    exs = [None] * nb

                    def scores(sb):
                        s0 = sb * 128
                        ps2 = psc.tile([128, 2, 512], F32, tag="ps_sc",
                                       bufs=OA_LAG, name="ps_sc")
                        for hi in range(2):
                            mm(out=ps2[:, hi, :],
                               lhsT=kT[hi * 64:hi * 64 + 64, hp, s0:s0 + 128],
                               rhs=qT[hi * 64:hi * 64 + 64, hp, c0:c0 + 512],
                               start=True, stop=True)
                        ex = ab.tile([128, 2, 512], BF, tag="ex",
                                     bufs=OA_LAG + 1, name="ex")
                        nc.scalar.activation(out=ex, in_=ps2, func=Act.Exp)
                        midx = sb - 4 * ci
                        if midx >= 0:
                            for hi in range(2):
                                nc.gpsimd.affine_select(
                                    out=ex[:, hi, :], in_=ex[:, hi, :],
                                    compare_op=Alu.is_ge, fill=0.0,
                                    base=-(midx * 128), channel_multiplier=-1,
                                    pattern=[[1, 512]])
                        exs[sb] = ex

                    def oacc(sb):
                        for hi in range(2):
                            h_loc = hp * 2 + hi
                            mm(out=po[hi],
                               lhsT=vS[:, sb, h_loc * 65:h_loc * 65 + 65],
                               rhs=exs[sb][:, hi, :],
                               start=(sb == 0), stop=(sb == nb - 1))

                    for step in range(nb + OA_LAG):
                        if step < nb:
                            scores(step)
                        if step >= OA_LAG:
                            oacc(step - OA_LAG)
                        stepctr += 1
                        if pending and stepctr % spacing == 0:
                            pending.pop(0)()

                    for hi in range(2):
                        h_loc = hp * 2 + hi
                        dnr = ab.tile([1, 512], F32, tag="dnr", bufs=2,
                                      name="dnr")
                        nc.vector.tensor_copy(out=dnr, in_=po[hi][64:65, :])
                        rcp = ab.tile([1, 512], F32, tag="rcp", bufs=2,
                                      name="rcp")
                        nc.vector.reciprocal_approx_fast(out=rcp, in_=dnr)
                        bc = ab.tile([64, 512], F32, tag="bc", bufs=2,
                                     name="bc")
                        nc.gpsimd.partition_broadcast(bc, rcp)
                        nc.vector.tensor_mul(
                            out=oT[hi * 64:hi * 64 + 64, hp, :],
                            in0=po[hi][0:64, :], in1=bc)
                while pending:
                    pending.pop(0)()

            def wo_units(ci, oT):
                rsv = rs_in[ci].rearrange("j (k p) t -> j k p t", p=128)
                units = []
                for dt in range(KD):
                    def u(dt=dt, oT=oT, rsv=rsv):
                        ps = pproj.tile([128, 512], F32, tag="ps_proj",
                                        bufs=2, name="ps_wo")
                        for k in range(KHE):
                            mm(out=ps,
                               lhsT=wo_sb[:, k, dt * 128:(dt + 1) * 128],
                               rhs=oT[:, k, :],
                               start=(k == 0), stop=(k == KHE - 1))
                        stg = ab.tile([128, 512], BF, tag="stg1", bufs=2,
                                      name="stg1")
                        nc.vector.tensor_scalar(
                            out=stg, in0=ps, scalar1=bo2_sb[:, dt:dt + 1],
                            scalar2=None, op0=Alu.add)
                        for j in range(TP):
                            nc.sync.dma_start(
                                out=rsv[j, dt, :, :],
                                in_=stg[:, j * 256:(j + 1) * 256])
                    units.append(u)
                return units

            def rs_issue(ci):
                nc.gpsimd.collective_compute(
                    "ReduceScatter", Alu.add, replica_groups=PAIRS,
                    ins=[rs_in[ci].opt()], outs=[rs_out[ci].opt()])

            hTs = {0: ln1(0)}
            for u in proj_units(0, hTs[0]):
                u()
            oTs = {}
            for ci in range(NCH):
                pending = []
                if ci >= 1:
                    pending += wo_units(ci - 1, oTs[ci - 1])
                if ci + 1 < NCH:
                    hTs[ci + 1] = ln1(ci + 1)
                    pending += proj_units(ci + 1, hTs[ci + 1])
                oTs[ci] = ab.tile([128, KHE, 512], BF, tag="oT", bufs=2,
                                  name="oT")
                attention(ci, oTs[ci], pending)
                if ci >= 1:
                    rs_issue(ci - 1)
            for u in wo_units(NCH - 1, oTs[NCH - 1]):
                u()
            rs_issue(NCH - 1)

    # ================= Stage B: residual + LN2 + FFN (own T/2 rows) ========
    with tc.tile_pool(name="de", bufs=1) as de, \
         tc.tile_pool(name="upsum", bufs=3, space="PSUM") as pu, \
         tc.tile_pool(name="fpsum", bufs=2, space="PSUM") as pf:

        w2_sb = de.tile([128, KFF, D], BF, tag="w2t", bufs=1, name="w2t")

        def prep(lc):
            c0 = lc * 512
            # residual: xmid = xres + rs_out  (bf16 residual stream)
            xr = de.tile([128, KD, 512], BF, tag="xr", bufs=1, name="xr")
            nc.gpsimd.dma_start(out=xr, in_=xres_v[:, :, c0:c0 + 512])
            arr = de.tile([128, KD, 2, 256], BF, tag="arr", bufs=1, name="arr")
            for j in range(2):
                nc.sync.dma_start(
                    out=arr[:, :, j, :],
                    in_=rs_out[2 * lc + j].rearrange("(k p) t -> p k t", p=128))
            xmid = de.tile([128, KD, 512], BF, tag="xmid", bufs=2,
                           name="xmid")
            for k in range(KD):
                nc.vector.tensor_add(
                    out=xmid[:, k, :], in0=xr[:, k, :],
                    in1=arr[:, k, :, :].rearrange("p j t -> p (j t)"))
            # LN2 (gains folded into W1/b1f on host)
            Ab2, Bb2 = _ln_stats(nc, de, pu, "ps_u", xmid, ones_col,
                                 psum_bufs=3)
            h2 = de.tile([128, KD, 512], BF, tag="h2", bufs=1, name="h2")
            for k in range(KD):
                _ln_apply(nc, de, xmid[:, k, :], Ab2, Bb2, h2[:, k, :])
            return xmid, h2

        def fc1(h2):
            # FFN up: u = relu(h2 @ W1 + b1f)   (relu+bias on DVE)
            u = de.tile([128, KFF, 512], BF, tag="u", bufs=1, name="u")
            for q16 in range(16):
                w1t = de.tile([128, KD, 256], BF, tag="w1t", bufs=2,
                              name="w1t")
                nc.sync.dma_start(out=w1t,
                                  in_=w1_v[:, :, q16 * 256:(q16 + 1) * 256])
                for fi in range(2):
                    fft = q16 * 2 + fi
                    ps = pu.tile([128, 512], F32, tag="ps_u", bufs=3,
                                 name="ps_u")
                    for k in range(KD):
                        mm(out=ps,
                           lhsT=w1t[:, k, fi * 128:fi * 128 + 128],
                           rhs=h2[:, k, :],
                           start=(k == 0), stop=(k == KD - 1))
                    nc.vector.tensor_scalar(
                        out=u[:, fft, :], in0=ps,
                        scalar1=b1_sb[:, fft:fft + 1], scalar2=0.0,
                        op0=Alu.add, op1=Alu.max)
            return u

        def fc2(lc, u, xmid):
            c0 = lc * 512
            # FFN down + bias + residual -> store
            for dt in range(KD):
                ps = pf.tile([128, 512], F32, tag="ps_f", bufs=2, name="ps_f")
                for k2 in range(KFF):
                    mm(out=ps,
                       lhsT=w2_sb[:, k2, dt * 128:(dt + 1) * 128],
                       rhs=u[:, k2, :],
                       start=(k2 == 0), stop=(k2 == KFF - 1))
                o_f = de.tile([128, 512], F32, tag="o_f", bufs=2, name="o_f")
                nc.vector.scalar_tensor_tensor(
                    out=o_f, in0=ps, scalar=b2_sb[:, dt:dt + 1],
                    in1=xmid[:, dt, :], op0=Alu.add, op1=Alu.add)
                nc.sync.dma_start(
                    out=outT_v[dt * 128:(dt + 1) * 128, c0:c0 + 512],
                    in_=o_f)

        xmid0, h20 = prep(0)
        u0 = fc1(h20)
        xmid1, h21 = prep(1)
        for q4 in range(4):
            nc.gpsimd.dma_start(
                out=w2_sb[:, q4 * 8:(q4 + 1) * 8, :],
                in_=w2_v[:, q4 * 8:(q4 + 1) * 8, :])
        fc2(0, u0, xmid0)
        u1 = fc1(h21)
        fc2(1, u1, xmid1)

    wlate.release()
    consts.release()
    dram.release()


def _build():
    nc = bacc.Bacc("TRN2", target_bir_lowering=False, debug=False,
                   num_devices=NCORES)

    tensors = {}
    tensors["xT"] = nc.dram_tensor("xT", [D, T], F32, kind="ExternalInput").ap()
    tensors["xresT"] = nc.dram_tensor("xresT", [D, LT], F32,
                                      kind="ExternalInput").ap()
    for name, shape, dt in (
        ("wq", [D, LHE], BF), ("wk", [D, LHE], BF), ("wv", [D, LHE], BF),
        ("wo", [LHE, D], BF), ("w1", [D, FF], BF), ("w2", [FF, D], BF),
        ("bq", [LHE], F32), ("bk", [LHE], F32), ("bv", [LHE], F32),
        ("b1f", [FF], F32), ("bo2", [D], F32), ("b2", [D], F32),
    ):
        tensors[name] = nc.dram_tensor(name, shape, dt,
                                       kind="ExternalInput").ap()
    tensors["outT"] = nc.dram_tensor("out", [D, LT], F32,
                                     kind="ExternalOutput").ap()

    with tile.TileContext(nc, num_cores=NCORES) as tc:
        _emit(nc, tc, tensors)

    nc.compile()
    return nc


_NC_CACHE = None


def _get_nc():
    global _NC_CACHE
    if _NC_CACHE is None:
        _NC_CACHE = _build()
    return _NC_CACHE


def _shard_inputs(x, Wq, Wk, Wv, Wo, bo, W1, b1, W2, b2, g1, be1, g2, be2):
    """Build the 8 per-core input maps (LN gains folded into weights)."""
    bf = lambda a: np.ascontiguousarray(a).astype(BF16NP)
    f32 = lambda a: np.ascontiguousarray(a, dtype=np.float32)

    x = np.asarray(x, dtype=np.float32)
    Wq = np.asarray(Wq, dtype=np.float32)
    Wk = np.asarray(Wk, dtype=np.float32)
    Wv = np.asarray(Wv, dtype=np.float32)
    Wo = np.asarray(Wo, dtype=np.float32)
    W1 = np.asarray(W1, dtype=np.float32)
    W2 = np.asarray(W2, dtype=np.float32)
    g1 = np.asarray(g1, dtype=np.float32)
    be1 = np.asarray(be1, dtype=np.float32)
    g2 = np.asarray(g2, dtype=np.float32)
    be2 = np.asarray(be2, dtype=np.float32)
    b1 = np.asarray(b1, dtype=np.float32)

    scale = float(HS) ** -0.5
    # fold g1 into QKV weights, be1 into QKV biases; fold the score scale
    # into Wq/bq.  Per-head [H, D, HS] -> concat heads -> [D, H*HS].
    wq_f = (g1[None, :, None] * Wq).transpose(1, 0, 2).reshape(D, D) * scale
    wk_f = (g1[None, :, None] * Wk).transpose(1, 0, 2).reshape(D, D)
    wv_f = (g1[None, :, None] * Wv).transpose(1, 0, 2).reshape(D, D)
    bq_f = np.einsum("d,hde->he", be1, Wq).reshape(D) * scale
    bk_f = np.einsum("d,hde->he", be1, Wk).reshape(D)
    bv_f = np.einsum("d,hde->he", be1, Wv).reshape(D)
    # fold g2/be2 into W1/b1
    w1_f = g2[:, None] * W1
    b1_f = b1 + be2 @ W1

    in_maps = []
    for c in range(NCORES):
        b, half = divmod(c, TP)
        hes = slice(half * LHE, (half + 1) * LHE)
        xt = x[b].T
        xres = np.concatenate(
            [xt[:, ci * 512 + half * 256: ci * 512 + half * 256 + 256]
             for ci in range(NCH)], axis=1)
        in_maps.append({
            "xT": f32(xt),
            "xresT": f32(xres),
            "wq": bf(wq_f[:, hes]), "wk": bf(wk_f[:, hes]),
            "wv": bf(wv_f[:, hes]),
            "bq": f32(bq_f[hes]), "bk": f32(bk_f[hes]), "bv": f32(bv_f[hes]),
            "wo": bf(Wo[hes, :]),
            "bo2": f32(np.asarray(bo, dtype=np.float32) / TP),
            "w1": bf(w1_f), "b1f": f32(b1_f),
            "w2": bf(W2), "b2": f32(np.asarray(b2, dtype=np.float32)),
        })
    return in_maps


def kernel(x, Wq, Wk, Wv, Wo, bo, W1, b1, W2, b2, g1, be1, g2, be2,
           _trace=False):
    nc = _get_nc()
    in_maps = _shard_inputs(x, Wq, Wk, Wv, Wo, bo, W1, b1, W2, b2,
                            g1, be1, g2, be2)
    res = run_bass_kernel_spmd(nc, in_maps, list(range(NCORES)),
                               trace=_trace)
    out = np.empty((B, T, D), dtype=np.float32)
    for b in range(B):
        for half in range(TP):
            o = res.results[TP * b + half]["out"]  # [D, LT]
            for ci in range(NCH):
                t0 = ci * 512 + half * 256
                out[b, t0:t0 + 256, :] = o[:, ci * 256:(ci + 1) * 256].T
    if _trace:
        kernel.last_exec_time_ns = res.exec_time_ns
        kernel.last_results = res
    return out
